# revision 4
# baseline (speedup 1.0000x reference)
"""LSTMCell forward on 8 Trainium2 NeuronCores (Bass/Tile, SPMD data-parallel).

Strategy (v3 = v2 transposed-z + stationary-weight reuse):
  - Shard the batch (32768) across 8 cores: 4096 rows each.
  - Compute z TRANSPOSED: z^T[n, b] = sum_k W[k, n] * xh[b, k].  Gate dim n
    (2048) on PSUM partitions in 16 chunks of 128; batch is the moving dim.
    lhsT = weight chunk [128k, 128n] (stationary), rhs = xh^T [128k, 512b].
  - All matmul inputs bf16 (same PE rate as fp32r, half the DMA, ~1e-2 rel).
  - Weight-stationary reuse: per (dc, gate, kc) the same lhsT feeds NCH=4
    adjacent matmuls (4 batch chunks of 512).  tile_legalize splits every
    matmul into LDWEIGHTS+MATMUL with no dedup; a post-pass rewrites the
    redundant LDWEIGHTS (identical weights AP, only matmuls in between) into
    sync-preserving NoOps -> 256 real weight loads instead of 1024.
  - Epilogue: ACT evacuates PSUM fused with the per-partition bias add
    (bias varies along partitions in this layout), DVE does the gate math in
    bf16, outputs written transposed in bf16 (host un-transposes + upcasts).
"""
import sys
from contextlib import nullcontext

if "/opt/trn_rl_repo" not in sys.path:
    sys.path.insert(0, "/opt/trn_rl_repo")

import numpy as np
import ml_dtypes
import concourse.bass as bass
import concourse.mybir as mybir
from concourse.tile import TileContext
from concourse.bass_utils import run_bass_kernel_spmd

F32 = mybir.dt.float32
BF16 = mybir.dt.bfloat16
AF = mybir.ActivationFunctionType
NP_BF16 = ml_dtypes.bfloat16

N_CORES = 8
P = 128
DH = 512
DH4 = 4 * DH            # 2048
K = 1024                # concat(x, h) contraction dim
KT = K // P             # 8 k-chunks
NDC = DH // P           # 4 d-chunks per gate
B_FULL = 32768
B_CORE = B_FULL // N_CORES   # 4096
CHUNK = 512                  # moving-dim width per matmul
NCH = 4                      # batch chunks per macro sharing one stationary
MACRO = CHUNK * NCH          # 2048
NMACRO = B_CORE // MACRO     # 2
DEDUP_LDWEIGHTS = True


def fanout_multi_waits(nc):
    """This walrus build rejects >1 sync wait per instruction: fan extra
    waits out onto single-wait NoOps on the same (in-order) engine."""
    n = 0
    for f in nc.m.functions:
        for bb in f.blocks:
            new = []
            for inst in bb.instructions:
                si = inst.sync_info
                waits = list(si.on_wait) if si and si.on_wait else []
                if len(waits) > 1:
                    for w in waits[:-1]:
                        nop = mybir.InstNoOp(name=f"waitfan_{n}", ins=[], outs=[])
                        n += 1
                        nop.engine = inst.engine
                        nop.sync_info = mybir.SyncInfo(on_wait=[w], on_update=[])
                        new.append(nop)
                    si.on_wait = [waits[-1]]
                new.append(inst)
            bb.instructions = new
    return n


def _ap_sig(arg):
    """Structural signature of a lowered AP argument (memory location +
    offset + access pattern + dtype)."""
    try:
        return repr(arg)
    except Exception:
        return None


def dedup_ldweights(nc):
    """Replace an InstLdweights whose weights AP is identical to the previous
    one on the PE engine (with only matmuls/noops in between) by a NoOp that
    preserves its semaphore waits/updates.  The PE array keeps the stationary
    across matmuls, so the reload is redundant (LDWEIGHTS + MATMUL xN)."""
    n = 0
    for f in nc.m.functions:
        for bb in f.blocks:
            last_sig = None
            for idx, inst in enumerate(bb.instructions):
                if inst.engine != mybir.EngineType.PE:
                    continue
                ty = type(inst).__name__
                if ty == "InstLdweights":
                    sig = _ap_sig(inst.ins[0]) if inst.ins else None
                    if sig is not None and sig == last_sig:
                        nop = mybir.InstNoOp(
                            name=f"lwdedup_{n}", ins=[], outs=[]
                        )
                        n += 1
                        nop.engine = inst.engine
                        nop.sync_info = inst.sync_info
                        bb.instructions[idx] = nop
                    else:
                        last_sig = sig
                elif ty in ("InstMatmult", "InstNoOp"):
                    continue
                else:
                    last_sig = None
    return n


def build_nc(loop_n=None):
    """Build the per-core program. loop_n wraps the body in a device-side
    For_i repeat (timing probe; outputs unchanged since the body is
    idempotent)."""
    nc = bass.Bass()
    # Pre-arranged on host: xhT[p, kc, b] = concat(x,h)[b, kc*128+p]
    xhT = nc.dram_tensor("xhT", [P, KT, B_CORE], BF16, kind="ExternalInput")
    # CT[p, dc, b] = C[b, dc*128+p]
    CT = nc.dram_tensor("CT", [P, NDC, B_CORE], BF16, kind="ExternalInput")
    # W[p, kc, n] = vstack(Wx, Wh)[kc*128+p, n]
    W = nc.dram_tensor("W", [P, KT, DH4], BF16, kind="ExternalInput")
    # bias[p, nc] = (bx+bh)[nc*128+p]
    bias = nc.dram_tensor("bias", [P, DH4 // P], F32, kind="ExternalInput")
    CnT = nc.dram_tensor("CnT", [P, NDC, B_CORE], BF16, kind="ExternalOutput")
    HnT = nc.dram_tensor("HnT", [P, NDC, B_CORE], BF16, kind="ExternalOutput")

    with TileContext(nc) as tc:
        with (
            tc.tile_pool(name="const", bufs=1) as const,
            tc.tile_pool(name="io", bufs=2) as io,
            tc.tile_pool(name="gates", bufs=2) as gates,
            tc.tile_pool(name="work", bufs=3) as work,
            tc.tile_pool(name="psum", bufs=2, space=bass.MemorySpace.PSUM) as psum,
        ):
            w_t = const.tile([P, KT, DH4], BF16)
            nc.sync.dma_start(out=w_t[:], in_=W[:])
            bias_t = const.tile([P, DH4 // P], F32)
            nc.sync.dma_start(out=bias_t[:], in_=bias[:])

            loop = tc.For_i(0, loop_n, 1) if loop_n else nullcontext()
            with loop:
                for mc in range(NMACRO):
                    bsl = slice(mc * MACRO, (mc + 1) * MACRO)
                    xh_t = io.tile([P, KT, NCH, CHUNK], BF16, tag="xh")
                    nc.sync.dma_start(out=xh_t[:], in_=xhT[:, :, bsl])
                    ct_t = io.tile([P, NDC, NCH, CHUNK], BF16, tag="ct")
                    nc.sync.dma_start(out=ct_t[:], in_=CT[:, :, bsl])

                    for dc in range(NDC):
                        # 4 gate tiles for this d-chunk, all batch chunks
                        gt4 = [
                            gates.tile(
                                [P, NCH, CHUNK], BF16, tag=f"g{g}",
                                name=f"gate{g}_{mc}_{dc}",
                            )
                            for g in range(4)
                        ]
                        for g, fn in enumerate(
                            [AF.Sigmoid, AF.Sigmoid, AF.Sigmoid, AF.Tanh]
                        ):
                            nci = g * NDC + dc
                            nsl = slice(nci * P, (nci + 1) * P)
                            zp = psum.tile([P, NCH, CHUNK], F32, tag="zp")
                            for kc in range(KT):
                                lhsT = w_t[:, kc, nsl]
                                for ch in range(NCH):
                                    nc.tensor.matmul(
                                        zp[:, ch, :],
                                        lhsT,
                                        xh_t[:, kc, ch, :],
                                        start=(kc == 0),
                                        stop=(kc == KT - 1),
                                    )
                            # ACT: evacuate PSUM + bias add + activation
                            for ch in range(NCH):
                                nc.scalar.activation(
                                    gt4[g][:, ch, :], zp[:, ch, :], fn,
                                    bias=bias_t[:, nci:nci + 1],
                                )
                        it, ft, ot, gg = gt4
                        cn_t = work.tile([P, NCH, CHUNK], BF16, tag="cn")
                        hn_t = work.tile([P, NCH, CHUNK], BF16, tag="hn")
                        for ch in range(NCH):
                            fc = work.tile([P, CHUNK], BF16, tag="fc")
                            nc.vector.tensor_mul(
                                fc[:], ft[:, ch, :], ct_t[:, dc, ch, :]
                            )
                            ig = work.tile([P, CHUNK], BF16, tag="ig")
                            nc.vector.tensor_mul(ig[:], it[:, ch, :], gg[:, ch, :])
                            nc.vector.tensor_add(cn_t[:, ch, :], fc[:], ig[:])
                            tch = work.tile([P, CHUNK], BF16, tag="tch")
                            nc.scalar.activation(tch[:], cn_t[:, ch, :], AF.Tanh)
                            nc.vector.tensor_mul(
                                hn_t[:, ch, :], ot[:, ch, :], tch[:]
                            )
                        nc.sync.dma_start(out=CnT[:, dc, bsl], in_=cn_t[:])
                        nc.sync.dma_start(out=HnT[:, dc, bsl], in_=hn_t[:])
    fanout_multi_waits(nc)
    if DEDUP_LDWEIGHTS:
        dedup_ldweights(nc)
    return nc


_NC = None


def _get_nc():
    global _NC
    if _NC is None:
        _NC = build_nc()
    return _NC


def make_in_maps(x, C, h, Wx, bx, Wh, bh):
    x = np.asarray(x, dtype=np.float32)
    C = np.asarray(C, dtype=np.float32)
    h = np.asarray(h, dtype=np.float32)
    Wfull = np.concatenate(
        [np.asarray(Wx, np.float32), np.asarray(Wh, np.float32)], axis=0
    )
    W_dr = np.ascontiguousarray(
        Wfull.reshape(KT, P, DH4).transpose(1, 0, 2)
    ).astype(NP_BF16)
    bias = np.asarray(bx, np.float32) + np.asarray(bh, np.float32)
    bias_dr = np.ascontiguousarray(bias.reshape(DH4 // P, P).T)
    in_maps = []
    for c in range(N_CORES):
        sl = slice(c * B_CORE, (c + 1) * B_CORE)
        xh = np.concatenate([x[sl], h[sl]], axis=1)          # [4096, 1024]
        xhT_dr = np.ascontiguousarray(
            xh.T.reshape(KT, P, B_CORE).transpose(1, 0, 2)
        ).astype(NP_BF16)
        CT_dr = np.ascontiguousarray(
            C[sl].T.reshape(NDC, P, B_CORE).transpose(1, 0, 2)
        ).astype(NP_BF16)
        in_maps.append(
            {"xhT": xhT_dr, "CT": CT_dr, "W": W_dr, "bias": bias_dr}
        )
    return in_maps


def _untranspose(arr):
    # [p, dc, b] -> [b, dc*128+p]
    return (
        np.asarray(arr).astype(np.float32).transpose(2, 1, 0).reshape(B_CORE, DH)
    )


def kernel(x, C, h, Wx, bx, Wh, bh):
    nc = _get_nc()
    in_maps = make_in_maps(x, C, h, Wx, bx, Wh, bh)
    res = run_bass_kernel_spmd(nc, in_maps, list(range(N_CORES)))
    C_new = np.concatenate(
        [_untranspose(res.results[c]["CnT"]) for c in range(N_CORES)], axis=0
    )
    h_new = np.concatenate(
        [_untranspose(res.results[c]["HnT"]) for c in range(N_CORES)], axis=0
    )
    return (C_new, h_new)
